# revision 35
# baseline (speedup 1.0000x reference)
"""Multi-head attention forward on 8 Trainium2 NeuronCores.

Computes, for x [16, 1024, 512], w_qkv [512, 1536], w_out [512, 512], b_out [512]:
    qkv = x @ w_qkv; q, k, v = split(qkv)
    out = softmax(q k^T / sqrt(512)) v          (8 heads, head_dim 64)
    return out @ w_out + b_out                  [16, 1024, 512]

Sharding: data-parallel over batch - 2 batches per core, no collectives.

Per-core design (v2, pipelined around the ScalarE exp floor):
  - The softmax exp is ~16.8M elements/core -> ~130us on ACT at 1 elem/lane/cyc.
    Everything else is arranged to hide underneath that.
  - Scores are computed transposed (s[j, i]) per head into [128, 2, 512] PSUM
    tiles drawn from a single rotating tag (bufs=2, 4 banks total), so ACT's
    1024-wide exps run back-to-back while PE writes the next tile.
  - exp outputs p in bf16 to SBUF; pv matmuls run bf16 with the contraction
    split into two K=64 row-group halves that execute concurrently on the PE.
  - v carries an appended ones column per head, so pv also emits the softmax
    denominator row (partition 64 of the [65, 512] psum output).
  - Normalization: reciprocal_approx_fast on the denominator row (~51 ULP),
    DMA partition-broadcast of the reciprocal row to 64 partitions, one DVE
    multiply into cT (bf16). Odd heads are shifted to partitions 64-127 of cT
    with a SBUF->SBUF DMA.
  - Software pipeline across batches: batch 1's qkv projection is emitted
    between batch 0's attention blocks (PSUM "gemm" pool shared with the
    output projection); batch 0's output projection is emitted between batch
    1's blocks and overlaps them. Projection evacuation copies run on ACT
    before the first exp and on DVE afterwards.
"""

import numpy as np

import concourse.bass as bass
from concourse import bacc
import concourse.mybir as mybir
import concourse.tile as tile
from concourse.bass_utils import run_bass_kernel_spmd

F32 = mybir.dt.float32
F32R = mybir.dt.float32r
BF16 = mybir.dt.bfloat16

N_CORES = 8
B = 16                 # global batch
BC = B // N_CORES      # batches per core
SEQ = 1024
TOK = BC * SEQ         # tokens per core
D = 512                # model dim
H = 8                  # heads
DH = D // H            # head dim = 64
SCALE = float(D) ** -0.5

PHASES = 3             # debug: 1=qkv proj only, 2=+attention, 3=full
REPEAT = 1             # debug: repeat whole kernel body (for timing differencing)
PV_SPLIT = False       # pv as two K=64 row-group matmuls: crashes HW (shared
                       # array-column accumulators between row groups) - keep off
NORM_DMA = True        # debug: broadcast reciprocal row via DRAM bounce
RECIP_FAST = False     # custom-DVE approx reciprocal: wrong results on HW in
                       # this run path (uop table not loaded) - keep off
P = 128                # partitions
KO = D // P            # 4 contraction chunks of 128
MT = TOK // P          # 16 token tiles of 128
JT = SEQ // P          # 8 key tiles per batch
NB = SEQ // 512        # 2 moving 512-token slabs per batch


def _r(ap):
    return ap.bitcast(F32R)


def _build_program():
    nc = bacc.Bacc("TRN2", target_bir_lowering=False, debug=False)

    x_d = nc.dram_tensor("xT", [D, TOK], F32R, kind="ExternalInput")
    wqkv_d = nc.dram_tensor("w_qkv", [D, 3 * D], F32R, kind="ExternalInput")
    wout_d = nc.dram_tensor("w_out", [D, D], F32, kind="ExternalInput")
    bout_d = nc.dram_tensor("b_out", [D], F32, kind="ExternalInput")
    out_d = nc.dram_tensor("out", [TOK, D], F32, kind="ExternalOutput")

    with tile.TileContext(nc) as tc:
        for _rep in range(REPEAT):
            _emit(tc, x_d.ap(), wqkv_d.ap(), wout_d.ap(), bout_d.ap(), out_d.ap())
    nc.compile()
    return nc


def _emit(tc, x_d, wqkv_d, wout_d, bout_d, out_d):
    nc = tc.nc
    Exp = mybir.ActivationFunctionType.Exp
    mult = mybir.AluOpType.mult
    add = mybir.AluOpType.add

    from contextlib import ExitStack
    with ExitStack() as ctx:
        persist = ctx.enter_context(tc.tile_pool(name="persist", bufs=1))

        # --- persistent tiles ---
        b_bc = persist.tile([P, D], F32)
        nc.sync.dma_start(out=b_bc, in_=bout_d.unsqueeze(0).to_broadcast((P, D)))
        w_out_sb = persist.tile([P, KO, D], BF16)
        with tc.tile_pool(name="wtmp", bufs=1) as wtmp:
            w_out_f32 = wtmp.tile([P, KO, D], F32)
            nc.sync.dma_start(
                out=w_out_f32, in_=wout_d.rearrange("(ko p) n -> p ko n", p=P)
            )
            nc.vector.tensor_copy(w_out_sb, w_out_f32)
        qkT = persist.tile([P, H, TOK], F32)           # do rows: q (0..511), k (512..1023)
        v_ext = persist.tile([P, MT, H, DH + 1], BF16)  # per tok-tile, per head: [v | 1]
        nc.vector.memset(v_ext[:, :, :, DH : DH + 1], 1.0)
        cT = [
            persist.tile([P, KO, SEQ], BF16, name=f"cT{i}") for i in range(BC)
        ]  # context^T, one per batch
        # scratch for the transposed-reciprocal: rows 64..95 hold data, the
        # pads are zeroed once so the 32x32 block transposes never read
        # uninitialized memory
        tpad = persist.tile([DH + 32, 512], F32)
        nc.vector.memset(tpad[DH : DH + 32, :], 0.0)
        rTp = persist.tile([DH + 32, 512], F32)
        nc.vector.memset(rTp[DH : DH + 32, :], 0.0)
        # warm the ACT exp table during the projection phase
        warm_sb = persist.tile([P, 2], F32)
        nc.vector.memset(warm_sb, 0.0)

        out_grp = out_d.rearrange("(t p) d -> t p d", p=P)
        if PHASES < 1:
            for t in range(MT):
                nc.sync.dma_start(out=out_grp[t], in_=b_bc)
            return

        early = ctx.enter_context(tc.tile_pool(name="early", bufs=1))
        w_qkv_sb = early.tile([P, KO, 3 * D], F32)
        # chunked weight loads: q/k columns first so projection can start
        # early; spread across engine DMA queues so they run in parallel
        wq_r = wqkv_d.rearrange("(ko p) n -> p ko n", p=P)
        nc.sync.dma_start(out=_r(w_qkv_sb[:, :, 0:D]), in_=wq_r[:, :, 0:D])
        nc.sync.dma_start(out=_r(w_qkv_sb[:, :, D : 2 * D]), in_=wq_r[:, :, D : 2 * D])
        nc.gpsimd.dma_start(
            out=_r(w_qkv_sb[:, :, 2 * D : 3 * D]), in_=wq_r[:, :, 2 * D : 3 * D]
        )
        xT = early.tile([P, KO, TOK], F32)
        x_r = x_d.rearrange("(c p) t -> p c t", p=P)
        nc.scalar.dma_start(out=_r(xT[:, :, 0:SEQ]), in_=x_r[:, :, 0:SEQ])
        nc.gpsimd.dma_start(out=_r(xT[:, :, SEQ:TOK]), in_=x_r[:, :, SEQ:TOK])

        nc.scalar.activation(warm_sb, warm_sb, Exp)  # load exp table early

        ps_gemm = ctx.enter_context(tc.tile_pool(name="ps_gemm", bufs=2, space="PSUM"))

        def proj_qk_chunk(b, mo, nt, on_act):
            """project one [128 dims x 512 tok] chunk of q or k for batch b."""
            ps = ps_gemm.tile([P, 512], F32, tag="gemm")
            t0 = b * SEQ + nt * 512
            for ko in range(KO):
                nc.tensor.matmul(
                    ps,
                    _r(w_qkv_sb[:, ko, mo * P : (mo + 1) * P]),
                    _r(xT[:, ko, t0 : t0 + 512]),
                    start=(ko == 0),
                    stop=(ko == KO - 1),
                )
            dst = _r(qkT[:, mo, t0 : t0 + 512])
            if on_act:
                nc.scalar.copy(dst, ps)
            else:
                nc.vector.tensor_copy(dst, ps)

        def proj_v_chunk(t, on_act):
            """project one 128-token tile of v (natural layout) into v_ext."""
            ps = ps_gemm.tile([P, 512], F32, tag="gemm")
            for ko in range(KO):
                nc.tensor.matmul(
                    ps,
                    _r(xT[:, ko, t * P : (t + 1) * P]),
                    _r(w_qkv_sb[:, ko, 2 * D : 3 * D]),
                    start=(ko == 0),
                    stop=(ko == KO - 1),
                )
            dst = v_ext[:, t, :, 0:DH]
            src = ps.rearrange("p (h d) -> p h d", h=H)
            if on_act:
                nc.scalar.copy(dst, src)
            else:
                nc.vector.tensor_copy(dst, src)

        def proj_chunks(b, on_act):
            """all projection work for batch b as a list of thunks."""
            thunks = []
            for mo in [0, 4, 1, 5, 2, 6, 3, 7]:
                for nt in range(NB):
                    thunks.append(
                        lambda b=b, mo=mo, nt=nt: proj_qk_chunk(b, mo, nt, on_act)
                    )
            for t in range(b * JT, (b + 1) * JT):
                thunks.append(lambda t=t: proj_v_chunk(t, on_act))
            return thunks

        # batch 0 projection head phase: emit only what the first blocks
        # need (q/k of head pair 0 + all of v); the rest is deferred into
        # the fill slots of the early attention blocks
        head_now, deferred_proj = [], []
        for mo in [0, 4]:
            for nt in range(NB):
                head_now.append(lambda mo=mo, nt=nt: proj_qk_chunk(0, mo, nt, True))
        for t in range(JT):
            head_now.append(lambda t=t: proj_v_chunk(t, True))
        for mo in [1, 5, 2, 6, 3, 7]:
            for nt in range(NB):
                deferred_proj.append(
                    lambda mo=mo, nt=nt: proj_qk_chunk(0, mo, nt, False)
                )
        for thunk in head_now:
            thunk()

        if PHASES < 2:
            for t in range(MT):
                nc.sync.dma_start(out=out_grp[t], in_=b_bc)
            return

        # =========== attention ===========
        def qT(h, b, ih):
            lo = DH * (h % 2)
            return qkT[lo : lo + DH, h // 2, b * SEQ + ih * 512 : b * SEQ + (ih + 1) * 512]

        def kT(h, b, jt):
            lo = DH * (h % 2)
            return qkT[lo : lo + DH, H // 2 + h // 2, b * SEQ + jt * P : b * SEQ + (jt + 1) * P]

        with (
            tc.tile_pool(name="p_sb", bufs=6) as p_sb,
            tc.tile_pool(name="norm_sb", bufs=3) as norm_sb,
            tc.tile_pool(name="o_sb", bufs=2) as o_pool,
            tc.tile_pool(name="stage_sb", bufs=4) as stage_sb,
            tc.tile_pool(name="ps_s", bufs=2, space="PSUM") as ps_s,
            tc.tile_pool(name="ps_o", bufs=1, space="PSUM") as ps_o,
            tc.tile_pool(name="dram_scr", bufs=4, space="DRAM") as dram_scr,
        ):
            def score_u_step(h, b, jt):
                """scores + exp for one head, one key tile, BOTH query slabs.

                One kT LDWEIGHTS feeds two back-to-back N=512 matmuls (the
                two query slabs), halving the weight-load count and letting
                consecutive matmuls overlap their pipeline drains.
                """
                s = ps_s.tile([P, 2, 512], F32, tag="s", name="s")
                for ih in range(2):
                    nc.tensor.matmul(
                        s[:, ih, :], _r(kT(h, b, jt)), _r(qT(h, b, ih))
                    )
                p = p_sb.tile([P, 2, 512], BF16, tag="p", name="p")
                nc.scalar.activation(p, s, Exp, scale=SCALE)
                return p

            def pv_u_step(h, b, jt, p, out0, out1):
                """pv for one key tile: one v LDWEIGHTS, both query slabs."""
                jg = b * JT + jt
                first = jt == 0
                last = jt == JT - 1
                nc.tensor.matmul(
                    out0[0 : DH + 1, :], v_ext[:, jg, h, :], p[:, 0, :],
                    start=first, stop=last,
                )
                nc.tensor.matmul(
                    out1[0 : DH + 1, :], v_ext[:, jg, h, :], p[:, 1, :],
                    start=first, stop=last,
                )

            def stage_out(outX):
                """evacuate [ctx | denom] from PSUM right after the last pv so
                the bank frees for the next block without waiting on norm."""
                st = stage_sb.tile([DH + 1, 512], F32, tag="st", name="st")
                nc.vector.tensor_copy(st, outX[0 : DH + 1, :])
                return st

            def norm_head(h, b, ih, st):
                """divide by the denominator row and store into cT[b]."""
                cols = slice(ih * 512, (ih + 1) * 512)
                rbc = norm_sb.tile([DH, 512], F32, tag="rbc", name="rbc")
                rrow = norm_sb.tile([DH + 32, 512], F32, tag="rrow", name="rrow")
                # reciprocal is ~8 cyc/free-dim-element regardless of
                # partition count, so spread the 512 denominators over
                # 32 partitions with a 32x32 block transpose: recip
                # then runs on 16 elements/lane instead of 512.
                nc.vector.tensor_copy(tpad[DH : DH + 1, :], st[DH : DH + 1, :])
                tT = norm_sb.tile([DH + 32, 512], F32, tag="tT", name="tT")
                nc.vector.transpose(tT[DH : DH + 32, :], tpad[DH : DH + 32, :])
                tT3 = tT[DH : DH + 32, :].rearrange("p (a b) -> p a b", b=32)
                rT3 = rTp[DH : DH + 32, :].rearrange("p (a b) -> p a b", b=32)
                nc.vector.reciprocal(rT3[:, :, 0:1], tT3[:, :, 0:1])
                nc.vector.transpose(rrow[DH : DH + 32, :], rTp[DH : DH + 32, :])
                # partition-broadcast the reciprocal row via a DRAM bounce
                # (SBUF-source partition-stride-0 DMAs are not supported)
                scr = dram_scr.tile([1, 512], F32, tag="scr", name="scr")
                nc.sync.dma_start(out=scr, in_=rrow[DH : DH + 1, :])
                nc.sync.dma_start(out=rbc, in_=scr.to_broadcast((DH, 512)))
                if h % 2 == 0:
                    nc.vector.tensor_tensor(
                        cT[b][0:DH, h // 2, cols], st[0:DH, :], rbc, mult
                    )
                else:
                    n_sb = norm_sb.tile([DH, 512], BF16, tag="n_sb", name="n_sb")
                    nc.vector.tensor_tensor(n_sb, st[0:DH, :], rbc, mult)
                    nc.sync.dma_start(
                        out=cT[b][DH:P, h // 2, cols], in_=n_sb
                    )

            def out_proj_tile(b, it):
                f_ps = ps_gemm.tile([P, D], F32, tag="gemm", name="f_ps")
                for ko in range(KO):
                    nc.tensor.matmul(
                        f_ps,
                        cT[b][:, ko, (it % JT) * P : (it % JT + 1) * P],
                        w_out_sb[:, ko, :],
                        start=(ko == 0),
                        stop=(ko == KO - 1),
                    )
                o_sb = o_pool.tile([P, D], F32, tag="o_sb", name="o_sb")
                nc.vector.tensor_tensor(o_sb, f_ps, b_bc, add)
                nc.sync.dma_start(out=out_grp[it], in_=o_sb)

            for b in range(BC):
                # deferred work interleaved between this batch's blocks:
                # remaining chunks of this batch's own projection (emitted
                # early so dependent blocks are never starved), the next
                # batch's projection, and the previous batch's out-projection
                fill = list(deferred_proj)
                if b + 1 < BC:
                    fill += proj_chunks(b + 1, on_act=False)
                if b > 0 and PHASES >= 3:
                    fill += [
                        lambda b=b, it=it: out_proj_tile(b - 1, it)
                        for it in range((b - 1) * JT, b * JT)
                    ]
                fill_i = 0

                def emit_fill(n):
                    nonlocal fill_i
                    for thunk in fill[fill_i : fill_i + n]:
                        thunk()
                    fill_i += n

                # pace the fill so it completes by slot 12 of 16 (leaves the
                # late blocks dense, and this batch's own deferred projection
                # chunks land well before the blocks that read them)
                n_slots = H * 2
                for h in range(H):
                    out0 = ps_o.tile([P, 512], F32, tag="out0", name="out0")
                    out1 = ps_o.tile([P, 512], F32, tag="out1", name="out1")
                    # software pipeline: pv trails scores by TWO key tiles so
                    # the PE is never queued behind an exp it must wait on,
                    # and the block-boundary pv (which waits on the previous
                    # block's staging copy) sits deep enough to be covered
                    hist = []
                    for jt in range(JT):
                        hist.append(score_u_step(h, b, jt))
                        if jt >= 4:
                            pv_u_step(h, b, jt - 4, hist[jt - 4], out0, out1)
                        elif jt % 2 == 0:
                            slot = h * 2 + jt // 2 + 1
                            emit_fill(
                                min(len(fill), (len(fill) * slot) // 12) - fill_i
                            )
                    for jt in range(JT - 4, JT):
                        pv_u_step(h, b, jt, hist[jt], out0, out1)
                    st0 = stage_out(out0)
                    st1 = stage_out(out1)
                    norm_head(h, b, 0, st0)
                    norm_head(h, b, 1, st1)
                emit_fill(len(fill) - fill_i)
                deferred_proj = []

            if PHASES < 3:
                return

            # last batch's output projection (tail; ACT is idle here)
            for it in range((BC - 1) * JT, BC * JT):
                out_proj_tile(BC - 1, it)


_CACHE = {}


def _get_nc():
    key = (PHASES, REPEAT)
    if key not in _CACHE:
        _CACHE[key] = _build_program()
    return _CACHE[key]


def round_f32r(a):
    """Round fp32 -> fp32r (sign, 8-bit exp, 11-bit stored mantissa), RTNE.

    The PE's fp32r datapath carries 20-bit floats; pre-rounding on the host
    makes the DMA'd operands exact fixed points of the hardware rounding.
    """
    u = np.ascontiguousarray(a, dtype=np.float32).view(np.uint32)
    lsb = (u >> 12) & 1
    u = (u + 0x7FF + lsb) & np.uint32(0xFFFFF000)
    return u.view(np.float32)


def run_sharded(inputs, **kw):
    """Run the SPMD kernel; returns (full_output [16,1024,512], BassKernelResults)."""
    nc = _get_nc()
    x = np.asarray(inputs["x"], dtype=np.float32)
    w_qkv = round_f32r(np.asarray(inputs["w_qkv"], dtype=np.float32))
    w_out = np.ascontiguousarray(np.asarray(inputs["w_out"], dtype=np.float32))
    b_out = np.ascontiguousarray(np.asarray(inputs["b_out"], dtype=np.float32))
    in_maps = [
        {
            "xT": round_f32r(
                np.ascontiguousarray(
                    x[c * BC : (c + 1) * BC].reshape(TOK, D).T
                )
            ),
            "w_qkv": w_qkv,
            "w_out": w_out,
            "b_out": b_out,
        }
        for c in range(N_CORES)
    ]
    res = run_bass_kernel_spmd(nc, in_maps, core_ids=list(range(N_CORES)), **kw)
    out = np.concatenate(
        [r["out"].reshape(BC, SEQ, D) for r in res.results], axis=0
    )
    return out, res


def kernel(x, w_qkv, w_out, b_out):
    out, _ = run_sharded(
        {"x": x, "w_qkv": w_qkv, "w_out": w_out, "b_out": b_out}
    )
    return out


# revision 36
# speedup vs baseline: 1.0142x; 1.0142x over previous
"""Multi-head attention forward on 8 Trainium2 NeuronCores.

Computes, for x [16, 1024, 512], w_qkv [512, 1536], w_out [512, 512], b_out [512]:
    qkv = x @ w_qkv; q, k, v = split(qkv)
    out = softmax(q k^T / sqrt(512)) v          (8 heads, head_dim 64)
    return out @ w_out + b_out                  [16, 1024, 512]

Sharding: data-parallel over batch - 2 batches per core, no collectives.

Per-core design (v2, pipelined around the ScalarE exp floor):
  - The softmax exp is ~16.8M elements/core -> ~130us on ACT at 1 elem/lane/cyc.
    Everything else is arranged to hide underneath that.
  - Scores are computed transposed (s[j, i]) per head into [128, 2, 512] PSUM
    tiles drawn from a single rotating tag (bufs=2, 4 banks total), so ACT's
    1024-wide exps run back-to-back while PE writes the next tile.
  - exp outputs p in bf16 to SBUF; pv matmuls run bf16 with the contraction
    split into two K=64 row-group halves that execute concurrently on the PE.
  - v carries an appended ones column per head, so pv also emits the softmax
    denominator row (partition 64 of the [65, 512] psum output).
  - Normalization: reciprocal_approx_fast on the denominator row (~51 ULP),
    DMA partition-broadcast of the reciprocal row to 64 partitions, one DVE
    multiply into cT (bf16). Odd heads are shifted to partitions 64-127 of cT
    with a SBUF->SBUF DMA.
  - Software pipeline across batches: batch 1's qkv projection is emitted
    between batch 0's attention blocks (PSUM "gemm" pool shared with the
    output projection); batch 0's output projection is emitted between batch
    1's blocks and overlaps them. Projection evacuation copies run on ACT
    before the first exp and on DVE afterwards.
"""

import numpy as np

import concourse.bass as bass
from concourse import bacc
import concourse.mybir as mybir
import concourse.tile as tile
from concourse.bass_utils import run_bass_kernel_spmd

F32 = mybir.dt.float32
F32R = mybir.dt.float32r
BF16 = mybir.dt.bfloat16

N_CORES = 8
B = 16                 # global batch
BC = B // N_CORES      # batches per core
SEQ = 1024
TOK = BC * SEQ         # tokens per core
D = 512                # model dim
H = 8                  # heads
DH = D // H            # head dim = 64
SCALE = float(D) ** -0.5

PHASES = 3             # debug: 1=qkv proj only, 2=+attention, 3=full
REPEAT = 1             # debug: repeat whole kernel body (for timing differencing)
PV_SPLIT = False       # pv as two K=64 row-group matmuls: crashes HW (shared
                       # array-column accumulators between row groups) - keep off
NORM_DMA = True        # debug: broadcast reciprocal row via DRAM bounce
RECIP_FAST = False     # custom-DVE approx reciprocal: wrong results on HW in
                       # this run path (uop table not loaded) - keep off
P = 128                # partitions
KO = D // P            # 4 contraction chunks of 128
MT = TOK // P          # 16 token tiles of 128
JT = SEQ // P          # 8 key tiles per batch
NB = SEQ // 512        # 2 moving 512-token slabs per batch


def _r(ap):
    return ap.bitcast(F32R)


def _build_program():
    nc = bacc.Bacc("TRN2", target_bir_lowering=False, debug=False)

    x_d = nc.dram_tensor("xT", [D, TOK], F32R, kind="ExternalInput")
    wqkv_d = nc.dram_tensor("w_qkv", [D, 3 * D], F32R, kind="ExternalInput")
    wout_d = nc.dram_tensor("w_out", [D, D], F32, kind="ExternalInput")
    bout_d = nc.dram_tensor("b_out", [D], F32, kind="ExternalInput")
    out_d = nc.dram_tensor("out", [TOK, D], F32, kind="ExternalOutput")

    with tile.TileContext(nc) as tc:
        for _rep in range(REPEAT):
            _emit(tc, x_d.ap(), wqkv_d.ap(), wout_d.ap(), bout_d.ap(), out_d.ap())
    nc.compile()
    return nc


def _emit(tc, x_d, wqkv_d, wout_d, bout_d, out_d):
    nc = tc.nc
    Exp = mybir.ActivationFunctionType.Exp
    mult = mybir.AluOpType.mult
    add = mybir.AluOpType.add

    from contextlib import ExitStack
    with ExitStack() as ctx:
        persist = ctx.enter_context(tc.tile_pool(name="persist", bufs=1))

        # --- persistent tiles ---
        b_bc = persist.tile([P, D], F32)
        nc.sync.dma_start(out=b_bc, in_=bout_d.unsqueeze(0).to_broadcast((P, D)))
        w_out_sb = persist.tile([P, KO, D], BF16)
        with tc.tile_pool(name="wtmp", bufs=1) as wtmp:
            w_out_f32 = wtmp.tile([P, KO, D], F32)
            nc.sync.dma_start(
                out=w_out_f32, in_=wout_d.rearrange("(ko p) n -> p ko n", p=P)
            )
            nc.vector.tensor_copy(w_out_sb, w_out_f32)
        qkT = persist.tile([P, H, TOK], F32)           # do rows: q (0..511), k (512..1023)
        v_ext = persist.tile([P, MT, H, DH + 1], BF16)  # per tok-tile, per head: [v | 1]
        nc.vector.memset(v_ext[:, :, :, DH : DH + 1], 1.0)
        cT = [
            persist.tile([P, KO, SEQ], BF16, name=f"cT{i}") for i in range(BC)
        ]  # context^T, one per batch
        # scratch for the transposed-reciprocal: rows 64..95 hold data, the
        # pads are zeroed once so the 32x32 block transposes never read
        # uninitialized memory
        tpad = persist.tile([DH + 32, 512], F32)
        nc.vector.memset(tpad[DH : DH + 32, :], 0.0)
        rTp = persist.tile([DH + 32, 512], F32)
        nc.vector.memset(rTp[DH : DH + 32, :], 0.0)
        # warm the ACT exp table during the projection phase
        warm_sb = persist.tile([P, 2], F32)
        nc.vector.memset(warm_sb, 0.0)

        out_grp = out_d.rearrange("(t p) d -> t p d", p=P)
        if PHASES < 1:
            for t in range(MT):
                nc.sync.dma_start(out=out_grp[t], in_=b_bc)
            return

        early = ctx.enter_context(tc.tile_pool(name="early", bufs=1))
        w_qkv_sb = early.tile([P, KO, 3 * D], F32)
        # chunked weight loads: q/k columns first so projection can start
        # early; spread across engine DMA queues so they run in parallel
        wq_r = wqkv_d.rearrange("(ko p) n -> p ko n", p=P)
        nc.sync.dma_start(out=_r(w_qkv_sb[:, :, 0:D]), in_=wq_r[:, :, 0:D])
        nc.sync.dma_start(out=_r(w_qkv_sb[:, :, D : 2 * D]), in_=wq_r[:, :, D : 2 * D])
        nc.gpsimd.dma_start(
            out=_r(w_qkv_sb[:, :, 2 * D : 3 * D]), in_=wq_r[:, :, 2 * D : 3 * D]
        )
        xT = early.tile([P, KO, TOK], F32)
        x_r = x_d.rearrange("(c p) t -> p c t", p=P)
        for ci, eng in enumerate((nc.scalar, nc.gpsimd, nc.scalar, nc.gpsimd)):
            eng.dma_start(
                out=_r(xT[:, :, ci * 512 : (ci + 1) * 512]),
                in_=x_r[:, :, ci * 512 : (ci + 1) * 512],
            )

        nc.scalar.activation(warm_sb, warm_sb, Exp)  # load exp table early

        ps_gemm = ctx.enter_context(tc.tile_pool(name="ps_gemm", bufs=2, space="PSUM"))

        def proj_qk_chunk(b, mo, nt, on_act):
            """project one [128 dims x 512 tok] chunk of q or k for batch b."""
            ps = ps_gemm.tile([P, 512], F32, tag="gemm")
            t0 = b * SEQ + nt * 512
            for ko in range(KO):
                nc.tensor.matmul(
                    ps,
                    _r(w_qkv_sb[:, ko, mo * P : (mo + 1) * P]),
                    _r(xT[:, ko, t0 : t0 + 512]),
                    start=(ko == 0),
                    stop=(ko == KO - 1),
                )
            dst = _r(qkT[:, mo, t0 : t0 + 512])
            if on_act:
                nc.scalar.copy(dst, ps)
            else:
                nc.vector.tensor_copy(dst, ps)

        def proj_v_chunk(t, on_act):
            """project one 128-token tile of v (natural layout) into v_ext."""
            ps = ps_gemm.tile([P, 512], F32, tag="gemm")
            for ko in range(KO):
                nc.tensor.matmul(
                    ps,
                    _r(xT[:, ko, t * P : (t + 1) * P]),
                    _r(w_qkv_sb[:, ko, 2 * D : 3 * D]),
                    start=(ko == 0),
                    stop=(ko == KO - 1),
                )
            dst = v_ext[:, t, :, 0:DH]
            src = ps.rearrange("p (h d) -> p h d", h=H)
            if on_act:
                nc.scalar.copy(dst, src)
            else:
                nc.vector.tensor_copy(dst, src)

        def proj_chunks(b, on_act):
            """all projection work for batch b as a list of thunks."""
            thunks = []
            for mo in [0, 4, 1, 5, 2, 6, 3, 7]:
                for nt in range(NB):
                    thunks.append(
                        lambda b=b, mo=mo, nt=nt: proj_qk_chunk(b, mo, nt, on_act)
                    )
            for t in range(b * JT, (b + 1) * JT):
                thunks.append(lambda t=t: proj_v_chunk(t, on_act))
            return thunks

        # batch 0 projection head phase: emit only what the first blocks
        # need (q/k of head pair 0 + all of v); the rest is deferred into
        # the fill slots of the early attention blocks
        head_now, deferred_proj = [], []
        for mo in [0, 4]:
            for nt in range(NB):
                head_now.append(lambda mo=mo, nt=nt: proj_qk_chunk(0, mo, nt, True))
        for t in range(JT):
            head_now.append(lambda t=t: proj_v_chunk(t, True))
        for mo in [1, 5, 2, 6, 3, 7]:
            for nt in range(NB):
                deferred_proj.append(
                    lambda mo=mo, nt=nt: proj_qk_chunk(0, mo, nt, False)
                )
        for thunk in head_now:
            thunk()

        if PHASES < 2:
            for t in range(MT):
                nc.sync.dma_start(out=out_grp[t], in_=b_bc)
            return

        # =========== attention ===========
        def qT(h, b, ih):
            lo = DH * (h % 2)
            return qkT[lo : lo + DH, h // 2, b * SEQ + ih * 512 : b * SEQ + (ih + 1) * 512]

        def kT(h, b, jt):
            lo = DH * (h % 2)
            return qkT[lo : lo + DH, H // 2 + h // 2, b * SEQ + jt * P : b * SEQ + (jt + 1) * P]

        with (
            tc.tile_pool(name="p_sb", bufs=6) as p_sb,
            tc.tile_pool(name="norm_sb", bufs=3) as norm_sb,
            tc.tile_pool(name="o_sb", bufs=2) as o_pool,
            tc.tile_pool(name="stage_sb", bufs=4) as stage_sb,
            tc.tile_pool(name="ps_s", bufs=2, space="PSUM") as ps_s,
            tc.tile_pool(name="ps_o", bufs=1, space="PSUM") as ps_o,
            tc.tile_pool(name="dram_scr", bufs=4, space="DRAM") as dram_scr,
        ):
            def score_pair_step(h1, h2, b, ih, jp):
                """scores + exp for both heads of a pair, one jp pair.

                The four score matmuls alternate between the two heads' PE
                row groups (0-63 / 64-127) so each LDWEIGHTS targets a
                different row group than the in-flight matmul and can be
                pulled ahead by the PE's reorder window.
                """
                sA = ps_s.tile([P, 2, 512], F32, tag="s", name="sA")
                sB = ps_s.tile([P, 2, 512], F32, tag="s", name="sB")
                for u in range(2):
                    nc.tensor.matmul(
                        sA[:, u, :], _r(kT(h1, b, 2 * jp + u)), _r(qT(h1, b, ih))
                    )
                    nc.tensor.matmul(
                        sB[:, u, :], _r(kT(h2, b, 2 * jp + u)), _r(qT(h2, b, ih))
                    )
                pA = p_sb.tile([P, 2, 512], BF16, tag="p", name="pA")
                nc.scalar.activation(pA, sA, Exp, scale=SCALE)
                pB = p_sb.tile([P, 2, 512], BF16, tag="p", name="pB")
                nc.scalar.activation(pB, sB, Exp, scale=SCALE)
                return pA, pB

            def pv_step(h, b, jp, p, outX):
                out = outX[0 : DH + 1, :]
                for u in range(2):
                    jg = b * JT + 2 * jp + u
                    nc.tensor.matmul(
                        out, v_ext[:, jg, h, :], p[:, u, :],
                        start=(jp == 0 and u == 0),
                        stop=(jp == JT // 2 - 1 and u == 1),
                    )

            def stage_out(outX):
                """evacuate [ctx | denom] from PSUM right after the last pv so
                the bank frees for the next block without waiting on norm."""
                st = stage_sb.tile([DH + 1, 512], F32, tag="st", name="st")
                nc.vector.tensor_copy(st, outX[0 : DH + 1, :])
                return st

            def norm_head(h, b, ih, st):
                """divide by the denominator row and store into cT[b]."""
                cols = slice(ih * 512, (ih + 1) * 512)
                rbc = norm_sb.tile([DH, 512], F32, tag="rbc", name="rbc")
                rrow = norm_sb.tile([DH + 32, 512], F32, tag="rrow", name="rrow")
                # reciprocal is ~8 cyc/free-dim-element regardless of
                # partition count, so spread the 512 denominators over
                # 32 partitions with a 32x32 block transpose: recip
                # then runs on 16 elements/lane instead of 512.
                nc.vector.tensor_copy(tpad[DH : DH + 1, :], st[DH : DH + 1, :])
                tT = norm_sb.tile([DH + 32, 512], F32, tag="tT", name="tT")
                nc.vector.transpose(tT[DH : DH + 32, :], tpad[DH : DH + 32, :])
                tT3 = tT[DH : DH + 32, :].rearrange("p (a b) -> p a b", b=32)
                rT3 = rTp[DH : DH + 32, :].rearrange("p (a b) -> p a b", b=32)
                nc.vector.reciprocal(rT3[:, :, 0:1], tT3[:, :, 0:1])
                nc.vector.transpose(rrow[DH : DH + 32, :], rTp[DH : DH + 32, :])
                # partition-broadcast the reciprocal row via a DRAM bounce
                # (SBUF-source partition-stride-0 DMAs are not supported)
                scr = dram_scr.tile([1, 512], F32, tag="scr", name="scr")
                nc.sync.dma_start(out=scr, in_=rrow[DH : DH + 1, :])
                nc.sync.dma_start(out=rbc, in_=scr.to_broadcast((DH, 512)))
                if h % 2 == 0:
                    nc.vector.tensor_tensor(
                        cT[b][0:DH, h // 2, cols], st[0:DH, :], rbc, mult
                    )
                else:
                    n_sb = norm_sb.tile([DH, 512], BF16, tag="n_sb", name="n_sb")
                    nc.vector.tensor_tensor(n_sb, st[0:DH, :], rbc, mult)
                    nc.sync.dma_start(
                        out=cT[b][DH:P, h // 2, cols], in_=n_sb
                    )

            def out_proj_tile(b, it):
                f_ps = ps_gemm.tile([P, D], F32, tag="gemm", name="f_ps")
                for ko in range(KO):
                    nc.tensor.matmul(
                        f_ps,
                        cT[b][:, ko, (it % JT) * P : (it % JT + 1) * P],
                        w_out_sb[:, ko, :],
                        start=(ko == 0),
                        stop=(ko == KO - 1),
                    )
                o_sb = o_pool.tile([P, D], F32, tag="o_sb", name="o_sb")
                nc.vector.tensor_tensor(o_sb, f_ps, b_bc, add)
                nc.sync.dma_start(out=out_grp[it], in_=o_sb)

            for b in range(BC):
                # deferred work interleaved between this batch's blocks:
                # remaining chunks of this batch's own projection (emitted
                # early so dependent blocks are never starved), the next
                # batch's projection, and the previous batch's out-projection
                fill = list(deferred_proj)
                if b + 1 < BC:
                    fill += proj_chunks(b + 1, on_act=False)
                if b > 0 and PHASES >= 3:
                    fill += [
                        lambda b=b, it=it: out_proj_tile(b - 1, it)
                        for it in range((b - 1) * JT, b * JT)
                    ]
                fill_i = 0

                def emit_fill(n):
                    nonlocal fill_i
                    for thunk in fill[fill_i : fill_i + n]:
                        thunk()
                    fill_i += n

                # pace the fill so it completes by slot 24 of 32 (leaves the
                # late blocks dense, and this batch's own deferred projection
                # chunks land well before the blocks that read them)
                for m in range(H // 2):
                    h1, h2 = 2 * m, 2 * m + 1
                    for ih in range(2):
                        blk = m * 2 + ih
                        outA = ps_o.tile([P, 512], F32, tag="outA", name="outA")
                        outB = ps_o.tile([P, 512], F32, tag="outB", name="outB")
                        # software pipeline: pv trails scores by TWO jp steps
                        # so the PE is never queued behind an exp it must wait
                        # on, and the block-boundary pv (which waits on the
                        # previous block's staging copy) sits deep enough to
                        # be covered
                        hist = []
                        for jp in range(JT // 2):
                            hist.append(score_pair_step(h1, h2, b, ih, jp))
                            if jp >= 2:
                                pv_step(h1, b, jp - 2, hist[jp - 2][0], outA)
                                pv_step(h2, b, jp - 2, hist[jp - 2][1], outB)
                            else:
                                slot = blk * 2 + jp + 1
                                emit_fill(
                                    min(len(fill), (len(fill) * slot) // 24)
                                    - fill_i
                                )
                        for jp in (JT // 2 - 2, JT // 2 - 1):
                            pv_step(h1, b, jp, hist[jp][0], outA)
                            pv_step(h2, b, jp, hist[jp][1], outB)
                        stA = stage_out(outA)
                        stB = stage_out(outB)
                        norm_head(h1, b, ih, stA)
                        norm_head(h2, b, ih, stB)
                emit_fill(len(fill) - fill_i)
                deferred_proj = []

            if PHASES < 3:
                return

            # last batch's output projection (tail; ACT is idle here)
            for it in range((BC - 1) * JT, BC * JT):
                out_proj_tile(BC - 1, it)


_CACHE = {}


def _get_nc():
    key = (PHASES, REPEAT)
    if key not in _CACHE:
        _CACHE[key] = _build_program()
    return _CACHE[key]


def round_f32r(a):
    """Round fp32 -> fp32r (sign, 8-bit exp, 11-bit stored mantissa), RTNE.

    The PE's fp32r datapath carries 20-bit floats; pre-rounding on the host
    makes the DMA'd operands exact fixed points of the hardware rounding.
    """
    u = np.ascontiguousarray(a, dtype=np.float32).view(np.uint32)
    lsb = (u >> 12) & 1
    u = (u + 0x7FF + lsb) & np.uint32(0xFFFFF000)
    return u.view(np.float32)


def run_sharded(inputs, **kw):
    """Run the SPMD kernel; returns (full_output [16,1024,512], BassKernelResults)."""
    nc = _get_nc()
    x = np.asarray(inputs["x"], dtype=np.float32)
    w_qkv = round_f32r(np.asarray(inputs["w_qkv"], dtype=np.float32))
    w_out = np.ascontiguousarray(np.asarray(inputs["w_out"], dtype=np.float32))
    b_out = np.ascontiguousarray(np.asarray(inputs["b_out"], dtype=np.float32))
    in_maps = [
        {
            "xT": round_f32r(
                np.ascontiguousarray(
                    x[c * BC : (c + 1) * BC].reshape(TOK, D).T
                )
            ),
            "w_qkv": w_qkv,
            "w_out": w_out,
            "b_out": b_out,
        }
        for c in range(N_CORES)
    ]
    res = run_bass_kernel_spmd(nc, in_maps, core_ids=list(range(N_CORES)), **kw)
    out = np.concatenate(
        [r["out"].reshape(BC, SEQ, D) for r in res.results], axis=0
    )
    return out, res


def kernel(x, w_qkv, w_out, b_out):
    out, _ = run_sharded(
        {"x": x, "w_qkv": w_qkv, "w_out": w_out, "b_out": b_out}
    )
    return out
